# revision 13
# baseline (speedup 1.0000x reference)
"""Trainium2 kernel for nn_Capture_Data: cap = sum(spec_data*filter, axis=(1,2))
plus Poisson/Gaussian noise synthesis.

Strategy:
  - The heavy, memory-bound part (reading 2 x 235MB and reducing over the 112
    (channel, spectral) slices) runs on 8 NeuronCores, one batch element per
    core (pure data parallel).
  - The accumulation is done with sequential fp32 adds in slice order 0..111,
    which reproduces XLA:CPU's column-reduction order bit-exactly.
  - The tiny noise-synthesis tail (jax.random.poisson/normal on the
    [8,256,256,1] result) is replicated with the exact same jax ops on the
    host CPU backend with threefry keys, matching the reference bit-for-bit.
"""

import numpy as np

P = 128             # SBUF partitions
CS = 112            # 4*28 reduced slices per batch element
NPIX = 256 * 256    # pixels per batch element
FREE = NPIX // P    # 512
G = 8               # slices loaded/multiplied per group
HEAD = 4            # leading single-slice groups (DVE starts early)
TAPER = 4           # trailing single-slice groups (tiny post-load tail)
N_CORES = 8

NL_IN = 0.5
CONS = 1e-10
POISSON_GAIN = 20.0

_TRACE = False      # set by test harness to collect an NTFF profile
LAST_EXEC_NS = None

_cached = {}


def _build_bass():
    import concourse.bacc as bacc
    import concourse.mybir as mybir
    from concourse.tile import TileContext

    nc = bacc.Bacc(None, target_bir_lowering=False)
    f32 = mybir.dt.float32
    spec = nc.dram_tensor("spec", [CS, NPIX], f32, kind="ExternalInput")
    filt = nc.dram_tensor("filt", [CS, NPIX], f32, kind="ExternalInput")
    cap = nc.dram_tensor("cap", [P, FREE], f32, kind="ExternalOutput")

    # Variable group sizes: small head groups so the DVE add chain starts
    # almost immediately; big groups in the steady state; tapered tail so
    # only ~2us of compute remains after the last load.
    groups = [1, 3, 4] + [G] * 13
    assert sum(groups) == CS

    with TileContext(nc) as tc:
        with (
            tc.tile_pool(name="io_s", bufs=4) as s_pool,
            tc.tile_pool(name="io_f", bufs=4) as f_pool,
            tc.tile_pool(name="io_p", bufs=2) as p_pool,
            tc.tile_pool(name="io_p1", bufs=4) as p1_pool,
            tc.tile_pool(name="accp", bufs=1) as acc_pool,
        ):
            acc = acc_pool.tile([P, FREE], f32)
            nc.vector.memset(acc[:], 0.0)
            cs0 = 0
            for gi, gs in enumerate(groups):
                w = gs * FREE
                # spec/filt share the big tags so loads are slot-paced by
                # mults (never by the add chain); single-slice tail groups
                # take their products from a small dedicated pool so the
                # add chain never gates a load either
                st = s_pool.tile([P, w], f32, tag="spec")
                ft = f_pool.tile([P, w], f32, tag="filt")
                if gs == 1:
                    prod = p1_pool.tile([P, w], f32, tag="prod1")
                else:
                    prod = p_pool.tile([P, w], f32, tag="prod")
                # slice k of this group lands at free-dim columns
                # [k*FREE, (k+1)*FREE) in the canonical pixel layout:
                # pixel = partition*FREE + i
                src_s = spec[cs0:cs0 + gs, :].rearrange("k (p i) -> p k i", p=P)
                src_f = filt[cs0:cs0 + gs, :].rearrange("k (p i) -> p k i", p=P)
                # two HWDGE rings (SP + ACT) so the two streams transfer in
                # parallel; alternate which ring carries which tensor so the
                # rings stay byte-balanced to the end of the stream
                eng_a, eng_b = (nc.sync, nc.scalar) if gi % 2 == 0 else (nc.scalar, nc.sync)
                eng_a.dma_start(out=st[:].rearrange("p (k i) -> p k i", k=gs), in_=src_s)
                eng_b.dma_start(out=ft[:].rearrange("p (k i) -> p k i", k=gs), in_=src_f)
                nc.vector.tensor_mul(prod[:], st[:], ft[:])
                # sequential accumulation in global slice order => matches
                # XLA:CPU reduction order bit-exactly
                for k in range(gs):
                    nc.vector.tensor_add(acc[:], acc[:], prod[:, k * FREE:(k + 1) * FREE])
                cs0 += gs

            nc.sync.dma_start(out=cap[:], in_=acc[:])
    nc.compile()
    return nc


def _run_device(spec_data, filt_data):
    """Run the Bass kernel on 8 cores; returns cap as float32 [8,256,256]."""
    global LAST_EXEC_NS
    from concourse.bass_utils import run_bass_kernel_spmd

    if "nc" not in _cached:
        _cached["nc"] = _build_bass()
    nc = _cached["nc"]

    in_maps = []
    for b in range(N_CORES):
        in_maps.append({
            "spec": np.ascontiguousarray(spec_data[b]).reshape(CS, NPIX),
            "filt": np.ascontiguousarray(filt_data[b]).reshape(CS, NPIX),
        })
    res = run_bass_kernel_spmd(nc, in_maps, list(range(N_CORES)), trace=_TRACE)
    LAST_EXEC_NS = res.exec_time_ns
    out = np.empty((N_CORES, 256, 256), dtype=np.float32)
    for b in range(N_CORES):
        out[b] = np.asarray(res.results[b]["cap"]).reshape(256, 256)
    return out


def _noise_synthesis(cap_np):
    """Replicates the reference's jax ops bit-exactly on the CPU backend."""
    import jax
    import jax.numpy as jnp

    cpu = jax.devices("cpu")[0]
    with jax.default_device(cpu):
        cap = jnp.asarray(cap_np)  # [8,256,256,1] float32
        poisson_t = jnp.full_like(cap, POISSON_GAIN * NL_IN)
        dark_t = jnp.full_like(cap, 1.0 * NL_IN)
        gauss_t = jnp.full_like(cap, 1.0 * NL_IN)
        peak = cap + CONS

        key = jax.random.key(1, impl="threefry2x32")
        kp, kd, kg = jax.random.split(key, 3)
        pnoisy = jax.random.poisson(kp, peak).astype(cap.dtype)
        dnoisy = jax.random.poisson(kd, dark_t).astype(cap.dtype)
        gnoisy = jax.random.normal(kg, cap.shape, dtype=cap.dtype) * gauss_t

        noisy = (pnoisy + dnoisy + gnoisy) * poisson_t / 255.0
        return (
            np.asarray(noisy),
            np.asarray(peak),
            np.asarray(dark_t),
            np.asarray(gauss_t ** 2),
        )


def kernel(spec_data, filter):
    spec_data = np.asarray(spec_data, dtype=np.float32)
    filt = np.asarray(filter, dtype=np.float32)
    cap = _run_device(spec_data, filt)[..., None]  # [8,256,256,1]
    return _noise_synthesis(cap)


# revision 14
# speedup vs baseline: 1.0127x; 1.0127x over previous
"""Trainium2 kernel for nn_Capture_Data: cap = sum(spec_data*filter, axis=(1,2))
plus Poisson/Gaussian noise synthesis.

Strategy:
  - The heavy, memory-bound part (reading 2 x 235MB and reducing over the 112
    (channel, spectral) slices) runs on 8 NeuronCores, one batch element per
    core (pure data parallel).
  - The accumulation is done with sequential fp32 adds in slice order 0..111,
    which reproduces XLA:CPU's column-reduction order bit-exactly.
  - The tiny noise-synthesis tail (jax.random.poisson/normal on the
    [8,256,256,1] result) is replicated with the exact same jax ops on the
    host CPU backend with threefry keys, matching the reference bit-for-bit.
"""

import numpy as np

P = 128             # SBUF partitions
CS = 112            # 4*28 reduced slices per batch element
NPIX = 256 * 256    # pixels per batch element
FREE = NPIX // P    # 512
G = 8               # slices loaded/multiplied per group
HEAD = 4            # leading single-slice groups (DVE starts early)
TAPER = 4           # trailing single-slice groups (tiny post-load tail)
N_CORES = 8

NL_IN = 0.5
CONS = 1e-10
POISSON_GAIN = 20.0

_TRACE = False      # set by test harness to collect an NTFF profile
LAST_EXEC_NS = None

_cached = {}


def _build_bass():
    import concourse.bacc as bacc
    import concourse.mybir as mybir
    from concourse.tile import TileContext

    nc = bacc.Bacc(None, target_bir_lowering=False)
    f32 = mybir.dt.float32
    spec = nc.dram_tensor("spec", [CS, NPIX], f32, kind="ExternalInput")
    filt = nc.dram_tensor("filt", [CS, NPIX], f32, kind="ExternalInput")
    cap = nc.dram_tensor("cap", [P, FREE], f32, kind="ExternalOutput")

    # Variable group sizes: small head groups so the DVE add chain starts
    # almost immediately; big groups in the steady state; tapered tail so
    # only ~2us of compute remains after the last load.
    groups = [1, 3, 4] + [G] * 13
    assert sum(groups) == CS

    with TileContext(nc) as tc:
        with (
            tc.tile_pool(name="io_s", bufs=4) as s_pool,
            tc.tile_pool(name="io_f", bufs=4) as f_pool,
            tc.tile_pool(name="io_p", bufs=2) as p_pool,
            tc.tile_pool(name="io_p1", bufs=4) as p1_pool,
            tc.tile_pool(name="accp", bufs=1) as acc_pool,
        ):
            acc = acc_pool.tile([P, FREE], f32)
            nc.vector.memset(acc[:], 0.0)
            cs0 = 0
            for gi, gs in enumerate(groups):
                w = gs * FREE
                # spec/filt share the big tags so loads are slot-paced by
                # mults (never by the add chain); single-slice tail groups
                # take their products from a small dedicated pool so the
                # add chain never gates a load either
                st = s_pool.tile([P, w], f32, tag="spec")
                ft = f_pool.tile([P, w], f32, tag="filt")
                if gs == 1:
                    prod = p1_pool.tile([P, w], f32, tag="prod1")
                else:
                    prod = p_pool.tile([P, w], f32, tag="prod")
                # slice k of this group lands at free-dim columns
                # [k*FREE, (k+1)*FREE) in the canonical pixel layout:
                # pixel = partition*FREE + i
                src_s = spec[cs0:cs0 + gs, :].rearrange("k (p i) -> p k i", p=P)
                src_f = filt[cs0:cs0 + gs, :].rearrange("k (p i) -> p k i", p=P)
                # two HWDGE rings (SP + ACT) so the two streams transfer in parallel
                nc.sync.dma_start(out=st[:].rearrange("p (k i) -> p k i", k=gs), in_=src_s)
                nc.scalar.dma_start(out=ft[:].rearrange("p (k i) -> p k i", k=gs), in_=src_f)
                nc.vector.tensor_mul(prod[:], st[:], ft[:])
                # sequential accumulation in global slice order => matches
                # XLA:CPU reduction order bit-exactly
                for k in range(gs):
                    nc.vector.tensor_add(acc[:], acc[:], prod[:, k * FREE:(k + 1) * FREE])
                cs0 += gs

            nc.sync.dma_start(out=cap[:], in_=acc[:])
    nc.compile()
    return nc


def _run_device(spec_data, filt_data):
    """Run the Bass kernel on 8 cores; returns cap as float32 [8,256,256]."""
    global LAST_EXEC_NS
    from concourse.bass_utils import run_bass_kernel_spmd

    if "nc" not in _cached:
        _cached["nc"] = _build_bass()
    nc = _cached["nc"]

    in_maps = []
    for b in range(N_CORES):
        in_maps.append({
            "spec": np.ascontiguousarray(spec_data[b]).reshape(CS, NPIX),
            "filt": np.ascontiguousarray(filt_data[b]).reshape(CS, NPIX),
        })
    res = run_bass_kernel_spmd(nc, in_maps, list(range(N_CORES)), trace=_TRACE)
    LAST_EXEC_NS = res.exec_time_ns
    out = np.empty((N_CORES, 256, 256), dtype=np.float32)
    for b in range(N_CORES):
        out[b] = np.asarray(res.results[b]["cap"]).reshape(256, 256)
    return out


def _noise_synthesis(cap_np):
    """Replicates the reference's jax ops bit-exactly on the CPU backend."""
    import jax
    import jax.numpy as jnp

    cpu = jax.devices("cpu")[0]
    with jax.default_device(cpu):
        cap = jnp.asarray(cap_np)  # [8,256,256,1] float32
        poisson_t = jnp.full_like(cap, POISSON_GAIN * NL_IN)
        dark_t = jnp.full_like(cap, 1.0 * NL_IN)
        gauss_t = jnp.full_like(cap, 1.0 * NL_IN)
        peak = cap + CONS

        key = jax.random.key(1, impl="threefry2x32")
        kp, kd, kg = jax.random.split(key, 3)
        pnoisy = jax.random.poisson(kp, peak).astype(cap.dtype)
        dnoisy = jax.random.poisson(kd, dark_t).astype(cap.dtype)
        gnoisy = jax.random.normal(kg, cap.shape, dtype=cap.dtype) * gauss_t

        noisy = (pnoisy + dnoisy + gnoisy) * poisson_t / 255.0
        return (
            np.asarray(noisy),
            np.asarray(peak),
            np.asarray(dark_t),
            np.asarray(gauss_t ** 2),
        )


def kernel(spec_data, filter):
    spec_data = np.asarray(spec_data, dtype=np.float32)
    filt = np.asarray(filter, dtype=np.float32)
    cap = _run_device(spec_data, filt)[..., None]  # [8,256,256,1]
    return _noise_synthesis(cap)


# revision 22
# speedup vs baseline: 1.0207x; 1.0079x over previous
"""Trainium2 kernel for nn_Capture_Data: cap = sum(spec_data*filter, axis=(1,2))
plus Poisson/Gaussian noise synthesis.

Strategy:
  - The heavy, memory-bound part (reading 2 x 235MB and reducing over the 112
    (channel, spectral) slices) runs on 8 NeuronCores, one batch element per
    core (pure data parallel).
  - The accumulation is done with sequential fp32 adds in slice order 0..111,
    which reproduces XLA:CPU's column-reduction order bit-exactly.
  - The tiny noise-synthesis tail (jax.random.poisson/normal on the
    [8,256,256,1] result) is replicated with the exact same jax ops on the
    host CPU backend with threefry keys, matching the reference bit-for-bit.
"""

import numpy as np

P = 128             # SBUF partitions
CS = 112            # 4*28 reduced slices per batch element
NPIX = 256 * 256    # pixels per batch element
FREE = NPIX // P    # 512
G = 8               # slices loaded/multiplied per group (steady state)
# head split so the DVE add chain starts early; small final group so little
# compute remains after the last load completes
GROUPS = [4, 4] + [G] * 12 + [6, 2]
N_CORES = 8

NL_IN = 0.5
CONS = 1e-10
POISSON_GAIN = 20.0

_TRACE = False      # set by test harness to collect an NTFF profile
LAST_EXEC_NS = None

_cached = {}


def _build_bass(groups=None, bufs_sf=4, bufs_p=2, pin_singles=True, bufs_1=8):
    import concourse.bacc as bacc
    import concourse.mybir as mybir
    from concourse.tile import TileContext
    from concourse.tile_rust import add_dep_helper

    nc = bacc.Bacc(None, target_bir_lowering=False)
    f32 = mybir.dt.float32
    spec = nc.dram_tensor("spec", [CS, NPIX], f32, kind="ExternalInput")
    filt = nc.dram_tensor("filt", [CS, NPIX], f32, kind="ExternalInput")
    cap = nc.dram_tensor("cap", [P, FREE], f32, kind="ExternalOutput")

    if groups is None:
        groups = list(GROUPS)
    assert sum(groups) == CS

    with TileContext(nc) as tc:
        with (
            tc.tile_pool(name="io_s", bufs=bufs_sf) as s_pool,
            tc.tile_pool(name="io_f", bufs=bufs_sf) as f_pool,
            tc.tile_pool(name="io_p", bufs=bufs_p) as p_pool,
            tc.tile_pool(name="io_1", bufs=bufs_1) as one_pool,
            tc.tile_pool(name="accp", bufs=1) as acc_pool,
        ):
            acc = acc_pool.tile([P, FREE], f32)
            nc.vector.memset(acc[:], 0.0)
            cs0 = 0
            last_big_s = last_big_f = None
            for gi, gs in enumerate(groups):
                w = gs * FREE
                if gs == 1:
                    # single-slice groups: dedicated small tiles so neither
                    # the mults nor the add chain gate their loads
                    st = one_pool.tile([P, w], f32, tag="s1")
                    ft = one_pool.tile([P, w], f32, tag="f1")
                    prod = one_pool.tile([P, w], f32, tag="p1")
                else:
                    st = s_pool.tile([P, w], f32, tag="spec")
                    ft = f_pool.tile([P, w], f32, tag="filt")
                    prod = p_pool.tile([P, w], f32, tag="prod")
                # slice k of this group lands at free-dim columns
                # [k*FREE, (k+1)*FREE) in the canonical pixel layout:
                # pixel = partition*FREE + i
                src_s = spec[cs0:cs0 + gs, :].rearrange("k (p i) -> p k i", p=P)
                src_f = filt[cs0:cs0 + gs, :].rearrange("k (p i) -> p k i", p=P)
                # two HWDGE rings (SP + ACT) so the two streams transfer in parallel
                dma_s = nc.sync.dma_start(out=st[:].rearrange("p (k i) -> p k i", k=gs), in_=src_s)
                dma_f = nc.scalar.dma_start(out=ft[:].rearrange("p (k i) -> p k i", k=gs), in_=src_f)
                if gs > 1:
                    last_big_s, last_big_f = dma_s, dma_f
                elif pin_singles and gi > 0 and last_big_s is not None:
                    # keep trailing single-slice loads from being hoisted
                    # early by the scheduler: order them after the last big
                    # group's loads so they arrive at the end of the stream
                    add_dep_helper(dma_s.ins, last_big_s.ins, sync=False,
                                   reason="tail single after big stream")
                    add_dep_helper(dma_f.ins, last_big_f.ins, sync=False,
                                   reason="tail single after big stream")
                nc.vector.tensor_mul(prod[:], st[:], ft[:])
                # sequential accumulation in global slice order => matches
                # XLA:CPU reduction order bit-exactly
                for k in range(gs):
                    nc.vector.tensor_add(acc[:], acc[:], prod[:, k * FREE:(k + 1) * FREE])
                cs0 += gs

            nc.sync.dma_start(out=cap[:], in_=acc[:])
    nc.compile()
    return nc


def _ensure_trace_hook_importable():
    """bass_utils imports antenv.axon_hooks when tracing is requested (e.g.
    BASS_TRACE=1 in the environment). Some images ship antenv without that
    submodule; provide a functional shim so a trace request degrades
    gracefully instead of crashing."""
    try:
        import antenv.axon_hooks  # noqa: F401
        return
    except ImportError:
        pass
    try:
        import sys
        import types

        mod = types.ModuleType("antenv.axon_hooks")
        mod._hook = None
        mod.set_axon_ntff_profile_hook = lambda h: setattr(mod, "_hook", h)
        # returning None makes bass_utils skip tracing gracefully
        mod.get_axon_ntff_profile_hook = lambda: mod._hook
        sys.modules["antenv.axon_hooks"] = mod
        import antenv
        antenv.axon_hooks = mod
    except Exception:
        pass


def _run_device(spec_data, filt_data):
    """Run the Bass kernel on 8 cores; returns cap as float32 [8,256,256]."""
    global LAST_EXEC_NS
    from concourse.bass_utils import run_bass_kernel_spmd

    _ensure_trace_hook_importable()

    if "nc" not in _cached:
        _cached["nc"] = _build_bass()
    nc = _cached["nc"]

    in_maps = []
    for b in range(N_CORES):
        in_maps.append({
            "spec": np.ascontiguousarray(spec_data[b]).reshape(CS, NPIX),
            "filt": np.ascontiguousarray(filt_data[b]).reshape(CS, NPIX),
        })
    res = run_bass_kernel_spmd(nc, in_maps, list(range(N_CORES)), trace=_TRACE)
    LAST_EXEC_NS = res.exec_time_ns
    out = np.empty((N_CORES, 256, 256), dtype=np.float32)
    for b in range(N_CORES):
        out[b] = np.asarray(res.results[b]["cap"]).reshape(256, 256)
    return out


def _noise_synthesis(cap_np):
    """Replicates the reference's jax ops bit-exactly on the CPU backend."""
    import jax
    import jax.numpy as jnp

    cpu = jax.devices("cpu")[0]
    with jax.default_device(cpu):
        cap = jnp.asarray(cap_np)  # [8,256,256,1] float32
        poisson_t = jnp.full_like(cap, POISSON_GAIN * NL_IN)
        dark_t = jnp.full_like(cap, 1.0 * NL_IN)
        gauss_t = jnp.full_like(cap, 1.0 * NL_IN)
        peak = cap + CONS

        key = jax.random.key(1, impl="threefry2x32")
        kp, kd, kg = jax.random.split(key, 3)
        pnoisy = jax.random.poisson(kp, peak).astype(cap.dtype)
        dnoisy = jax.random.poisson(kd, dark_t).astype(cap.dtype)
        gnoisy = jax.random.normal(kg, cap.shape, dtype=cap.dtype) * gauss_t

        noisy = (pnoisy + dnoisy + gnoisy) * poisson_t / 255.0
        return (
            np.asarray(noisy),
            np.asarray(peak),
            np.asarray(dark_t),
            np.asarray(gauss_t ** 2),
        )


def kernel(spec_data, filter):
    spec_data = np.asarray(spec_data, dtype=np.float32)
    filt = np.asarray(filter, dtype=np.float32)
    cap = _run_device(spec_data, filt)[..., None]  # [8,256,256,1]
    return _noise_synthesis(cap)
